# revision 13
# baseline (speedup 1.0000x reference)
"""Masked attention kernel for Trainium2, 8 NeuronCores.

Problem: q,k,v [32,1024,64] f32, mask [32,1024,1024] bool (True -> -inf),
out = softmax(q@k^T * D^-0.5 masked) @ v.

Sharding: batch*head dim (32) split across 8 cores, 4 heads/core.

Per-core device algorithm (T-layout), v2 -- engine-balanced:
  scoresT[t,s] = sum_d k[t,d] q[s,d]  via PE row-group pairs (qkT
      host-duplicated into partitions 64-127, two t-tiles concurrent).
  mask, split by tile to balance engines (ACT exp is the ~37us floor;
      PE and DVE are tucked underneath it):
        tiles 5-7 of each head: additive on PE in PSUM,
            scoresT += (-240*I128) @ mT  (mask fp8, drop=1 encoding)
        tiles 0-4: multiplicative on DVE after exp,
            pT *= keepT  (keep=1-mask, fp8, 1x tensor_tensor)
  pT = exp(0.125 * scoresT) on ACT (no row-max: |0.125 s| <= ~6).
  outT_aug[d,s] = sum_t v_aug[t,d] pT[t,s], v_aug = [v | ones] so row 64
      carries softmax denominators; accumulated in s-halves of 512 so the
      PSUM budget fits 3 score slots (ACT never starves).
  tail: outT+sums transposed together via 65-row PE transposes (the sums
      become column 64 of each transposed block), reciprocal runs on a
      strided [128,8] view, division is one broadcast tensor_mul.
PSUM budget (8 banks): 3 score slots (6) + o_ps half (1) + f_ps (1).
PE is HAM-warmed with dummy matmuls during the initial DMA so real QK
matmuls run at 2.4 GHz. All DRAM tensors host pre-tiled so every DMA is
a straight 128-partition contiguous copy. Host does only layout work:
transposes/casts/tiling of inputs+outputs.
"""

import os
import sys

import numpy as np

for _p in ("/opt/trn_rl_repo", "/opt/pypackages"):
    if _p not in sys.path and os.path.isdir(_p):
        sys.path.append(_p)

import ml_dtypes  # noqa: E402

import concourse.bass as bass  # noqa: E402
import concourse.tile as tile  # noqa: E402
from concourse import mybir  # noqa: E402
from concourse.bass_utils import run_bass_kernel_spmd  # noqa: E402

BH, S, D = 32, 1024, 64
NCORES = 8
HPC = BH // NCORES  # heads per core
NT = S // 128  # 8 tiles of 128 along s/t
FP8 = ml_dtypes.float8_e4m3fn
F32 = mybir.dt.float32
BF16 = mybir.dt.bfloat16
DT8 = mybir.dt.float8e4
MASK_NEG = -240.0  # exp(0.125*(-240)) ~ 9e-14; representable in fp8e4m3
PE_MASK_TILES = (5, 6, 7)  # these t-tiles get the additive PE mask path


def _build_program():
    nc = bass.Bass(
        "TRN2",
        target_bir_lowering=False,
        debug=False,
        num_devices=NCORES,
    )
    qkt = nc.dram_tensor("qkt", [HPC, 128, 2 * S], BF16, kind="ExternalInput").ap()
    vaug = nc.dram_tensor("vaug", [HPC, 128, NT * 65], BF16, kind="ExternalInput").ap()
    mt8 = nc.dram_tensor("mt8", [HPC, 128, NT * S], DT8, kind="ExternalInput").ap()
    negi = nc.dram_tensor("negi", [128, 128], DT8, kind="ExternalInput").ap()
    ident = nc.dram_tensor("ident", [65, 65], BF16, kind="ExternalInput").ap()
    outp = nc.dram_tensor("outp", [HPC, 128, NT * D], F32, kind="ExternalOutput").ap()

    with tile.TileContext(nc) as tc:
        with (
            tc.tile_pool(name="const", bufs=1) as const_pool,
            tc.tile_pool(name="qk", bufs=HPC) as qk_pool,
            tc.tile_pool(name="v", bufs=HPC) as v_pool,
            tc.tile_pool(name="m", bufs=HPC) as m_pool,
            tc.tile_pool(name="p", bufs=2) as p_pool,
            tc.tile_pool(name="ot", bufs=2) as ot_pool,
            tc.tile_pool(name="fin", bufs=2) as fin_pool,
            tc.tile_pool(name="spsum", bufs=3, space="PSUM") as s_pool,
            tc.tile_pool(name="opsum", bufs=1, space="PSUM") as o_pool,
        ):
            negi_sb = const_pool.tile([128, 128], DT8)
            nc.sync.dma_start(negi_sb[:], negi[:])
            ident_sb = const_pool.tile([65, 65], BF16)
            nc.sync.dma_start(ident_sb[:], ident[:])
            warm_sb = const_pool.tile([1, 1], F32)
            nc.gpsimd.memset(warm_sb[:], 0.0)
            warm_out = const_pool.tile([1, 1], F32, tag="warmo")
            nc.scalar.activation(
                out=warm_out[:],
                in_=warm_sb[:],
                func=mybir.ActivationFunctionType.Exp,
            )
            # HAM warmup: keep the PE busy during the initial input DMA so the
            # clock gate is at 8/8 (2.4 GHz) when the real matmuls arrive.
            warm_ps = s_pool.tile([128, 128], F32, name="warm_ps", tag="sc")
            for i in range(28):
                nc.tensor.matmul(
                    out=warm_ps[:],
                    lhsT=negi_sb[:],
                    rhs=negi_sb[:],
                    start=(i == 0),
                    stop=(i == 27),
                )

            qk_tiles, v_tiles, m_tiles = [], [], []
            for h in range(HPC):
                qk_sb = qk_pool.tile([128, 2 * S], BF16)
                m_sb = m_pool.tile([128, NT * S], DT8)
                v_sb = v_pool.tile([128, NT * 65], BF16)
                if h == 0:
                    # priority-ordered start: qT + first kT tiles, then v (AV
                    # runs within the head now), then the DVE-mask tiles, the
                    # rest of k, and finally the PE-mask tiles
                    nc.sync.dma_start(qk_sb[:, : S + 256], qkt[h][:, : S + 256])
                    nc.sync.dma_start(v_sb[:], vaug[h])
                    q1 = 3 * S
                    nc.sync.dma_start(m_sb[:, :q1], mt8[h][:, :q1])
                    nc.sync.dma_start(qk_sb[:, S + 256 :], qkt[h][:, S + 256 :])
                    nc.sync.dma_start(m_sb[:, q1:], mt8[h][:, q1:])
                else:
                    nc.sync.dma_start(qk_sb[:], qkt[h])
                    nc.sync.dma_start(v_sb[:], vaug[h])
                    nc.sync.dma_start(m_sb[:], mt8[h])
                qk_tiles.append(qk_sb)
                v_tiles.append(v_sb)
                m_tiles.append(m_sb)

            p_tiles = {}

            def emit_pair(h, pr):
                """QK + mask + exp for t-tiles (2*pr, 2*pr+1) of head h."""
                qk_sb, m_sb = qk_tiles[h], m_tiles[h]
                p_sb = p_tiles[h]
                scs = []
                for i in (0, 1):
                    t = 2 * pr + i
                    rows = slice(64 * i, 64 * i + 64)
                    kslc = slice(S + t * 128, S + (t + 1) * 128)
                    sc = s_pool.tile([128, S], F32, tag="sc")
                    pe_masked = t in PE_MASK_TILES
                    for n in range(2):
                        sl = slice(n * 512, (n + 1) * 512)
                        nc.tensor.matmul(
                            out=sc[:, sl],
                            lhsT=qk_sb[rows, kslc],
                            rhs=qk_sb[rows, sl],
                            start=True,
                            stop=not pe_masked,
                        )
                    scs.append((t, sc, pe_masked))
                for t, sc, pe_masked in scs:
                    if pe_masked:
                        for n in range(2):
                            sl = slice(n * 512, (n + 1) * 512)
                            nc.tensor.matmul(
                                out=sc[:, sl],
                                lhsT=negi_sb[:],
                                rhs=m_sb[:, t * S + n * 512 : t * S + (n + 1) * 512],
                                start=False,
                                stop=True,
                            )
                for t, sc, pe_masked in scs:
                    psl = slice(t * S, (t + 1) * S)
                    nc.scalar.activation(
                        out=p_sb[:, psl],
                        in_=sc[:],
                        func=mybir.ActivationFunctionType.Exp,
                        scale=0.125,
                    )
                    if not pe_masked:
                        nc.vector.tensor_mul(
                            out=p_sb[:, psl],
                            in0=p_sb[:, psl],
                            in1=m_sb[:, psl],
                        )

            av_state = {}

            def emit_av_chunk(h, pr):
                """AV t-tiles (2*pr, 2*pr+1) of head h into its [65,1024]
                accumulator (each matmul targets one 512-f32 bank)."""
                v_sb = v_tiles[h]
                p_sb = p_tiles[h]
                if pr == 0:
                    o_ps = o_pool.tile([65, S], F32, name="o_ps")
                    av_state[h] = o_ps
                else:
                    o_ps = av_state[h]
                for i in (0, 1):
                    t = 2 * pr + i
                    for n in range(2):
                        nc.tensor.matmul(
                            out=o_ps[:, n * 512 : (n + 1) * 512],
                            lhsT=v_sb[:, t * 65 : (t + 1) * 65],
                            rhs=p_sb[:, t * S + n * 512 : t * S + (n + 1) * 512],
                            start=(t == 0),
                            stop=(t == NT - 1),
                        )

            def emit_tail(h):
                """Drain AV, transpose outT(+sums) to [s,d], divide, DMA."""
                o_ps = av_state.pop(h)
                ot_sb = ot_pool.tile([65, S], BF16, name="ot_sb")
                nc.vector.tensor_copy(ot_sb[:], o_ps[:])
                # f shares the score-slot rotation (tag "sc"); 66-wide blocks
                # keep each bf16 transpose output 4B-aligned
                f_ps = s_pool.tile([128, NT * 66], BF16, name="f_ps", tag="sc")
                for j in range(NT):
                    nc.tensor.transpose(
                        out=f_ps[:, j * 66 : j * 66 + 65],
                        in_=ot_sb[:, j * 128 : (j + 1) * 128],
                        identity=ident_sb[:],
                    )
                f3 = f_ps[:].rearrange("p (j c) -> p j c", j=NT)
                r_sb = fin_pool.tile([128, NT], F32, tag="rsb")
                nc.vector.reciprocal(r_sb[:], f3[:, :, 64])
                out_sb = fin_pool.tile([128, NT * D], F32, tag="osb")
                nc.vector.tensor_mul(
                    out=out_sb[:].rearrange("p (j d) -> p j d", j=NT),
                    in0=f3[:, :, 0:64],
                    in1=r_sb[:, :, None].to_broadcast((128, NT, D)),
                )
                nc.sync.dma_start(outp[h], out_sb[:])

            # AV chunks lag their pair by one so the PE never waits on the
            # freshly-written p tiles; the tail drains right after the head.
            for h in range(HPC):
                p_tiles[h] = p_pool.tile([128, NT * S], BF16, name="p_sb", tag="p")
                for pr in range(4):
                    emit_pair(h, pr)
                    if pr > 0:
                        emit_av_chunk(h, pr - 1)
                emit_av_chunk(h, 3)
                emit_tail(h)
                p_tiles.pop(h - 1, None)

    _dedupe_ldweights(nc)
    _split_multi_waits(nc)
    return nc


def _dedupe_ldweights(nc):
    """Bass emits one InstLdweights per matmul; the PE keeps its weight state
    between matmuls, so a reload of the exact same weights AP with only
    matmuls/semaphores in between is pure overhead (~P/1.2 ns each). Drop the
    repeats, preserving their sync conditions via bare EventSemaphores. Data
    hazards stay tracked: the InstMatmult itself carries the weights AP read,
    so the tile framework's semaphore graph is unaffected."""
    for bb in nc.bb_map.values():
        insts = bb.bb.instructions
        new_list = []
        last_key = None
        for inst in insts:
            tn = type(inst).__name__
            eng = getattr(inst, "engine", None)
            if eng != mybir.EngineType.PE:
                new_list.append(inst)
                continue
            if tn == "InstLdweights":
                key = (repr(inst.ins[0]), bool(inst.is_transpose))
                if key == last_key:
                    si = getattr(inst, "sync_info", None)
                    if si is not None and (si.on_wait or si.on_update):
                        new_list.append(
                            mybir.InstEventSemaphore(
                                name=nc.get_next_instruction_name(),
                                ins=[],
                                outs=[],
                                engine=inst.engine,
                                sync_info=si,
                            )
                        )
                    continue
                last_key = key
            elif tn == "InstMatmult":
                if getattr(inst, "is_transpose", False):
                    last_key = None
            elif tn != "InstEventSemaphore":
                last_key = None
            new_list.append(inst)
        insts[:] = new_list


def _split_multi_waits(nc):
    """Walrus's S3_LW codegen can't take >1 sync-wait condition on a Matmult;
    hoist extras into standalone EventSemaphore instructions (same semantics:
    the engine queue stalls on them in program order, like raw-bass wait_ge)."""
    for bb in nc.bb_map.values():
        insts = bb.bb.instructions
        new_list = []
        for inst in insts:
            si = getattr(inst, "sync_info", None)
            if (
                si is not None
                and si.on_wait
                and len(si.on_wait) > 1
            ):
                extra = si.on_wait[:-1]
                keep = si.on_wait[-1:]
                for cond in extra:
                    new_list.append(
                        mybir.InstEventSemaphore(
                            name=nc.get_next_instruction_name(),
                            ins=[],
                            outs=[],
                            engine=inst.engine,
                            sync_info=mybir.SyncInfo(on_wait=[cond], on_update=[]),
                        )
                    )
                si.on_wait = keep
            new_list.append(inst)
        insts[:] = new_list


import concourse.bass_utils as _bu

_orig_run_command = _bu.run_command


# note: --enable-ldw-opt=true is unusable here -- walrus rejects the
# standalone InstLdweights that bass emits for every matmul.

_NC_CACHE = None


def _get_nc():
    global _NC_CACHE
    if _NC_CACHE is None:
        _NC_CACHE = _build_program()
    return _NC_CACHE


def _make_in_maps(q, k, v, mask):
    q = np.ascontiguousarray(np.asarray(q, dtype=np.float32))
    k = np.ascontiguousarray(np.asarray(k, dtype=np.float32))
    v = np.ascontiguousarray(np.asarray(v, dtype=np.float32))
    mask = np.asarray(mask)
    negi_np = (np.eye(128, dtype=np.float32) * MASK_NEG).astype(FP8)
    ident_np = np.eye(65, dtype=ml_dtypes.bfloat16)
    ones_col = np.ones((HPC, S, 1), dtype=np.float32)
    in_maps = []
    for c in range(NCORES):
        sl = slice(c * HPC, (c + 1) * HPC)
        qT = q[sl].transpose(0, 2, 1)  # [HPC, 64, S]
        kT = k[sl].transpose(0, 2, 1)
        qk1 = np.concatenate([qT, kT], axis=2)  # [HPC, 64, 2S]
        qkt_np = np.ascontiguousarray(
            np.concatenate([qk1, qk1], axis=1)
        ).astype(ml_dtypes.bfloat16)  # rows duplicated for PE row-group packing
        va = np.concatenate([v[sl], ones_col], axis=2)  # [HPC, S, 65]
        vaug_np = np.ascontiguousarray(
            va.reshape(HPC, NT, 128, 65).transpose(0, 2, 1, 3).reshape(HPC, 128, NT * 65)
        ).astype(ml_dtypes.bfloat16)
        mT = mask[sl].transpose(0, 2, 1).astype(np.float32)  # [HPC, t=S, s=S]
        mt = mT.reshape(HPC, NT, 128, S).transpose(0, 2, 1, 3)  # [HPC,128,NT,S]
        # tiles 0-4: DVE multiplicative path wants keep = 1-mask
        mt_enc = mt.copy()
        for t in range(NT):
            if t not in PE_MASK_TILES:
                mt_enc[:, :, t, :] = 1.0 - mt[:, :, t, :]
        mt8_np = np.ascontiguousarray(mt_enc.reshape(HPC, 128, NT * S)).astype(FP8)
        in_maps.append(
            {
                "qkt": qkt_np,
                "vaug": vaug_np,
                "mt8": mt8_np,
                "negi": negi_np,
                "ident": ident_np,
            }
        )
    return in_maps


def _gather(results):
    outs = []
    for c in range(NCORES):
        o = np.asarray(results[c]["outp"], dtype=np.float32)  # [HPC,128,NT*D]
        o = o.reshape(HPC, 128, NT, D).transpose(0, 2, 1, 3).reshape(HPC, S, D)
        outs.append(o)
    return np.ascontiguousarray(np.concatenate(outs, axis=0))


def _install_profile_shim():
    """The agent image's antenv lacks axon_hooks; recreate it from the boot
    module's ctypes implementation so trace=True can capture NTFF profiles."""
    import types

    if "antenv.axon_hooks" in sys.modules:
        return
    try:
        from trn_agent_boot.trn_boot import _ntff_profile_via_ctypes

        hook = _ntff_profile_via_ctypes("/opt/axon/libaxon_pjrt.so")
        mod = types.ModuleType("antenv.axon_hooks")
        mod.get_axon_ntff_profile_hook = lambda: hook
        mod.set_axon_ntff_profile_hook = lambda h: None
        sys.modules["antenv.axon_hooks"] = mod
        # don't try to copy artifacts to a remote bucket from the sandbox
        import concourse.bass_utils as _bu

        _bu.upload_artifacts = lambda tmpdir: tmpdir
    except Exception as e:  # profiling is best-effort
        print(f"profile shim unavailable: {e}", file=sys.stderr)


def run(q, k, v, mask, trace=False, **kw):
    nc = _get_nc()
    if trace:
        _install_profile_shim()
    in_maps = _make_in_maps(q, k, v, mask)
    res = run_bass_kernel_spmd(nc, in_maps, list(range(NCORES)), trace=trace, **kw)
    return _gather(res.results), res


def kernel(q, k, v, mask):
    out, _ = run(q, k, v, mask)
    return out


# revision 14
# speedup vs baseline: 1.1909x; 1.1909x over previous
"""Masked attention kernel for Trainium2, 8 NeuronCores.

Problem: q,k,v [32,1024,64] f32, mask [32,1024,1024] bool (True -> -inf),
out = softmax(q@k^T * D^-0.5 masked) @ v.

Sharding: batch*head dim (32) split across 8 cores, 4 heads/core.

Per-core device algorithm (T-layout), v2 -- engine-balanced:
  scoresT[t,s] = sum_d k[t,d] q[s,d]  via PE row-group pairs (qkT
      host-duplicated into partitions 64-127, two t-tiles concurrent).
  mask, split by tile to balance engines (ACT exp is the ~37us floor;
      PE and DVE are tucked underneath it):
        tiles 5-7 of each head: additive on PE in PSUM,
            scoresT += (-240*I128) @ mT  (mask fp8, drop=1 encoding)
        tiles 0-4: multiplicative on DVE after exp,
            pT *= keepT  (keep=1-mask, fp8, 1x tensor_tensor)
  pT = exp(0.125 * scoresT) on ACT (no row-max: |0.125 s| <= ~6).
  outT_aug[d,s] = sum_t v_aug[t,d] pT[t,s], v_aug = [v | ones] so row 64
      carries softmax denominators; accumulated in s-halves of 512 so the
      PSUM budget fits 3 score slots (ACT never starves).
  tail: outT+sums transposed together via 65-row PE transposes (the sums
      become column 64 of each transposed block), reciprocal runs on a
      strided [128,8] view, division is one broadcast tensor_mul.
PSUM budget (8 banks): 3 score slots (6) + o_ps half (1) + f_ps (1).
PE is HAM-warmed with dummy matmuls during the initial DMA so real QK
matmuls run at 2.4 GHz. All DRAM tensors host pre-tiled so every DMA is
a straight 128-partition contiguous copy. Host does only layout work:
transposes/casts/tiling of inputs+outputs.
"""

import os
import sys

import numpy as np

for _p in ("/opt/trn_rl_repo", "/opt/pypackages"):
    if _p not in sys.path and os.path.isdir(_p):
        sys.path.append(_p)

import ml_dtypes  # noqa: E402

import concourse.bass as bass  # noqa: E402
import concourse.tile as tile  # noqa: E402
from concourse import mybir  # noqa: E402
from concourse.bass_utils import run_bass_kernel_spmd  # noqa: E402

BH, S, D = 32, 1024, 64
NCORES = 8
HPC = BH // NCORES  # heads per core
NT = S // 128  # 8 tiles of 128 along s/t
FP8 = ml_dtypes.float8_e4m3fn
F32 = mybir.dt.float32
BF16 = mybir.dt.bfloat16
DT8 = mybir.dt.float8e4
MASK_NEG = -240.0  # exp(0.125*(-240)) ~ 9e-14; representable in fp8e4m3
PE_MASK_TILES = (5, 6, 7)  # these t-tiles get the additive PE mask path


def _build_program():
    nc = bass.Bass(
        "TRN2",
        target_bir_lowering=False,
        debug=False,
        num_devices=NCORES,
    )
    qkt = nc.dram_tensor("qkt", [HPC, 128, 2 * S], BF16, kind="ExternalInput").ap()
    vaug = nc.dram_tensor("vaug", [HPC, 128, NT * 65], BF16, kind="ExternalInput").ap()
    mt8 = nc.dram_tensor("mt8", [HPC, 128, NT * S], DT8, kind="ExternalInput").ap()
    negi = nc.dram_tensor("negi", [128, 128], DT8, kind="ExternalInput").ap()
    ident = nc.dram_tensor("ident", [65, 65], BF16, kind="ExternalInput").ap()
    outp = nc.dram_tensor("outp", [HPC, 128, NT * D], F32, kind="ExternalOutput").ap()

    with tile.TileContext(nc) as tc:
        with (
            tc.tile_pool(name="const", bufs=1) as const_pool,
            tc.tile_pool(name="qk", bufs=HPC) as qk_pool,
            tc.tile_pool(name="v", bufs=HPC) as v_pool,
            tc.tile_pool(name="m", bufs=HPC) as m_pool,
            tc.tile_pool(name="p", bufs=2) as p_pool,
            tc.tile_pool(name="ot", bufs=2) as ot_pool,
            tc.tile_pool(name="fin", bufs=2) as fin_pool,
            tc.tile_pool(name="spsum", bufs=3, space="PSUM") as s_pool,
            tc.tile_pool(name="opsum", bufs=1, space="PSUM") as o_pool,
        ):
            negi_sb = const_pool.tile([128, 128], DT8)
            nc.sync.dma_start(negi_sb[:], negi[:])
            ident_sb = const_pool.tile([65, 65], BF16)
            nc.sync.dma_start(ident_sb[:], ident[:])
            warm_sb = const_pool.tile([1, 1], F32)
            nc.gpsimd.memset(warm_sb[:], 0.0)
            warm_out = const_pool.tile([1, 1], F32, tag="warmo")
            nc.scalar.activation(
                out=warm_out[:],
                in_=warm_sb[:],
                func=mybir.ActivationFunctionType.Exp,
            )
            # HAM warmup: keep the PE busy during the initial input DMA so the
            # clock gate is at 8/8 (2.4 GHz) when the real matmuls arrive.
            warm_ps = s_pool.tile([128, 128], F32, name="warm_ps", tag="sc")
            for i in range(28):
                nc.tensor.matmul(
                    out=warm_ps[:],
                    lhsT=negi_sb[:],
                    rhs=negi_sb[:],
                    start=(i == 0),
                    stop=(i == 27),
                )

            qk_tiles, v_tiles, m_tiles = [], [], []
            for h in range(HPC):
                qk_sb = qk_pool.tile([128, 2 * S], BF16)
                m_sb = m_pool.tile([128, NT * S], DT8)
                v_sb = v_pool.tile([128, NT * 65], BF16)
                if h == 0:
                    # priority-ordered start: qT + first kT tiles, then v (AV
                    # runs within the head now), then the DVE-mask tiles, the
                    # rest of k, and finally the PE-mask tiles
                    nc.sync.dma_start(qk_sb[:, : S + 256], qkt[h][:, : S + 256])
                    nc.sync.dma_start(v_sb[:], vaug[h])
                    q1 = 3 * S
                    nc.sync.dma_start(m_sb[:, :q1], mt8[h][:, :q1])
                    nc.sync.dma_start(qk_sb[:, S + 256 :], qkt[h][:, S + 256 :])
                    nc.sync.dma_start(m_sb[:, q1:], mt8[h][:, q1:])
                else:
                    nc.sync.dma_start(qk_sb[:], qkt[h])
                    nc.sync.dma_start(v_sb[:], vaug[h])
                    nc.sync.dma_start(m_sb[:], mt8[h])
                qk_tiles.append(qk_sb)
                v_tiles.append(v_sb)
                m_tiles.append(m_sb)

            p_tiles = {}

            def emit_pair(h, pr):
                """QK + mask + exp for t-tiles (2*pr, 2*pr+1) of head h."""
                qk_sb, m_sb = qk_tiles[h], m_tiles[h]
                p_sb = p_tiles[h]
                scs = []
                for i in (0, 1):
                    t = 2 * pr + i
                    rows = slice(64 * i, 64 * i + 64)
                    kslc = slice(S + t * 128, S + (t + 1) * 128)
                    sc = s_pool.tile([128, S], F32, tag="sc")
                    pe_masked = t in PE_MASK_TILES
                    for n in range(2):
                        sl = slice(n * 512, (n + 1) * 512)
                        nc.tensor.matmul(
                            out=sc[:, sl],
                            lhsT=qk_sb[rows, kslc],
                            rhs=qk_sb[rows, sl],
                            start=True,
                            stop=not pe_masked,
                        )
                    scs.append((t, sc, pe_masked))
                for t, sc, pe_masked in scs:
                    if pe_masked:
                        for n in range(2):
                            sl = slice(n * 512, (n + 1) * 512)
                            nc.tensor.matmul(
                                out=sc[:, sl],
                                lhsT=negi_sb[:],
                                rhs=m_sb[:, t * S + n * 512 : t * S + (n + 1) * 512],
                                start=False,
                                stop=True,
                            )
                for t, sc, pe_masked in scs:
                    psl = slice(t * S, (t + 1) * S)
                    nc.scalar.activation(
                        out=p_sb[:, psl],
                        in_=sc[:],
                        func=mybir.ActivationFunctionType.Exp,
                        scale=0.125,
                    )
                    if not pe_masked:
                        nc.vector.tensor_mul(
                            out=p_sb[:, psl],
                            in0=p_sb[:, psl],
                            in1=m_sb[:, psl],
                        )

            av_state = {}

            def emit_av_chunk(h, pr):
                """AV t-tiles (2*pr, 2*pr+1) of head h into its [65,1024]
                accumulator (each matmul targets one 512-f32 bank)."""
                v_sb = v_tiles[h]
                p_sb = p_tiles[h]
                if pr == 0:
                    o_ps = o_pool.tile([65, S], F32, name="o_ps")
                    av_state[h] = o_ps
                else:
                    o_ps = av_state[h]
                for i in (0, 1):
                    t = 2 * pr + i
                    for n in range(2):
                        nc.tensor.matmul(
                            out=o_ps[:, n * 512 : (n + 1) * 512],
                            lhsT=v_sb[:, t * 65 : (t + 1) * 65],
                            rhs=p_sb[:, t * S + n * 512 : t * S + (n + 1) * 512],
                            start=(t == 0),
                            stop=(t == NT - 1),
                        )

            def emit_tail(h):
                """Drain AV, transpose outT(+sums) to [s,d], divide, DMA."""
                o_ps = av_state.pop(h)
                ot_sb = ot_pool.tile([65, S], BF16, name="ot_sb")
                nc.vector.tensor_copy(ot_sb[:], o_ps[:])
                # f shares the score-slot rotation (tag "sc"); 66-wide blocks
                # keep each bf16 transpose output 4B-aligned
                f_ps = s_pool.tile([128, NT * 66], BF16, name="f_ps", tag="sc")
                for j in range(NT):
                    nc.tensor.transpose(
                        out=f_ps[:, j * 66 : j * 66 + 65],
                        in_=ot_sb[:, j * 128 : (j + 1) * 128],
                        identity=ident_sb[:],
                    )
                f3 = f_ps[:].rearrange("p (j c) -> p j c", j=NT)
                r_sb = fin_pool.tile([128, NT], F32, tag="rsb")
                nc.vector.reciprocal(r_sb[:], f3[:, :, 64])
                out_sb = fin_pool.tile([128, NT * D], F32, tag="osb")
                nc.vector.tensor_mul(
                    out=out_sb[:].rearrange("p (j d) -> p j d", j=NT),
                    in0=f3[:, :, 0:64],
                    in1=r_sb[:, :, None].to_broadcast((128, NT, D)),
                )
                nc.sync.dma_start(outp[h], out_sb[:])

            # AV chunks lag their pair by one so the PE never waits on the
            # freshly-written p tiles; the tail drains right after the head.
            for h in range(HPC):
                p_tiles[h] = p_pool.tile([128, NT * S], BF16, name="p_sb", tag="p")
                for pr in range(4):
                    emit_pair(h, pr)
                    if pr > 0:
                        emit_av_chunk(h, pr - 1)
                emit_av_chunk(h, 3)
                emit_tail(h)
                p_tiles.pop(h - 1, None)

    if os.environ.get("KERNEL_DEDUPE_LDW", "0") == "1":
        _dedupe_ldweights(nc)
    _split_multi_waits(nc)
    return nc


def _dedupe_ldweights(nc):
    """Bass emits one InstLdweights per matmul; the PE keeps its weight state
    between matmuls, so a reload of the exact same weights AP with only
    matmuls/semaphores in between is pure overhead (~P/1.2 ns each). Drop the
    repeats, preserving their sync conditions via bare EventSemaphores. Data
    hazards stay tracked: the InstMatmult itself carries the weights AP read,
    so the tile framework's semaphore graph is unaffected."""
    for bb in nc.bb_map.values():
        insts = bb.bb.instructions
        new_list = []
        last_key = None
        for inst in insts:
            tn = type(inst).__name__
            eng = getattr(inst, "engine", None)
            if eng != mybir.EngineType.PE:
                new_list.append(inst)
                continue
            if tn == "InstLdweights":
                key = (repr(inst.ins[0]), bool(inst.is_transpose))
                if key == last_key:
                    si = getattr(inst, "sync_info", None)
                    if si is not None and (si.on_wait or si.on_update):
                        new_list.append(
                            mybir.InstEventSemaphore(
                                name=nc.get_next_instruction_name(),
                                ins=[],
                                outs=[],
                                engine=inst.engine,
                                sync_info=si,
                            )
                        )
                    continue
                last_key = key
            elif tn == "InstMatmult":
                if getattr(inst, "is_transpose", False):
                    last_key = None
            elif tn != "InstEventSemaphore":
                last_key = None
            new_list.append(inst)
        insts[:] = new_list


def _split_multi_waits(nc):
    """Walrus's S3_LW codegen can't take >1 sync-wait condition on a Matmult;
    hoist extras into standalone EventSemaphore instructions (same semantics:
    the engine queue stalls on them in program order, like raw-bass wait_ge)."""
    for bb in nc.bb_map.values():
        insts = bb.bb.instructions
        new_list = []
        for inst in insts:
            si = getattr(inst, "sync_info", None)
            if (
                si is not None
                and si.on_wait
                and len(si.on_wait) > 1
            ):
                extra = si.on_wait[:-1]
                keep = si.on_wait[-1:]
                for cond in extra:
                    new_list.append(
                        mybir.InstEventSemaphore(
                            name=nc.get_next_instruction_name(),
                            ins=[],
                            outs=[],
                            engine=inst.engine,
                            sync_info=mybir.SyncInfo(on_wait=[cond], on_update=[]),
                        )
                    )
                si.on_wait = keep
            new_list.append(inst)
        insts[:] = new_list


import concourse.bass_utils as _bu

_orig_run_command = _bu.run_command


# note: --enable-ldw-opt=true is unusable here -- walrus rejects the
# standalone InstLdweights that bass emits for every matmul.

_NC_CACHE = None


def _get_nc():
    global _NC_CACHE
    if _NC_CACHE is None:
        _NC_CACHE = _build_program()
    return _NC_CACHE


def _make_in_maps(q, k, v, mask):
    q = np.ascontiguousarray(np.asarray(q, dtype=np.float32))
    k = np.ascontiguousarray(np.asarray(k, dtype=np.float32))
    v = np.ascontiguousarray(np.asarray(v, dtype=np.float32))
    mask = np.asarray(mask)
    negi_np = (np.eye(128, dtype=np.float32) * MASK_NEG).astype(FP8)
    ident_np = np.eye(65, dtype=ml_dtypes.bfloat16)
    ones_col = np.ones((HPC, S, 1), dtype=np.float32)
    in_maps = []
    for c in range(NCORES):
        sl = slice(c * HPC, (c + 1) * HPC)
        qT = q[sl].transpose(0, 2, 1)  # [HPC, 64, S]
        kT = k[sl].transpose(0, 2, 1)
        qk1 = np.concatenate([qT, kT], axis=2)  # [HPC, 64, 2S]
        qkt_np = np.ascontiguousarray(
            np.concatenate([qk1, qk1], axis=1)
        ).astype(ml_dtypes.bfloat16)  # rows duplicated for PE row-group packing
        va = np.concatenate([v[sl], ones_col], axis=2)  # [HPC, S, 65]
        vaug_np = np.ascontiguousarray(
            va.reshape(HPC, NT, 128, 65).transpose(0, 2, 1, 3).reshape(HPC, 128, NT * 65)
        ).astype(ml_dtypes.bfloat16)
        mT = mask[sl].transpose(0, 2, 1).astype(np.float32)  # [HPC, t=S, s=S]
        mt = mT.reshape(HPC, NT, 128, S).transpose(0, 2, 1, 3)  # [HPC,128,NT,S]
        # tiles 0-4: DVE multiplicative path wants keep = 1-mask
        mt_enc = mt.copy()
        for t in range(NT):
            if t not in PE_MASK_TILES:
                mt_enc[:, :, t, :] = 1.0 - mt[:, :, t, :]
        mt8_np = np.ascontiguousarray(mt_enc.reshape(HPC, 128, NT * S)).astype(FP8)
        in_maps.append(
            {
                "qkt": qkt_np,
                "vaug": vaug_np,
                "mt8": mt8_np,
                "negi": negi_np,
                "ident": ident_np,
            }
        )
    return in_maps


def _gather(results):
    outs = []
    for c in range(NCORES):
        o = np.asarray(results[c]["outp"], dtype=np.float32)  # [HPC,128,NT*D]
        o = o.reshape(HPC, 128, NT, D).transpose(0, 2, 1, 3).reshape(HPC, S, D)
        outs.append(o)
    return np.ascontiguousarray(np.concatenate(outs, axis=0))


def _install_profile_shim():
    """The agent image's antenv lacks axon_hooks; recreate it from the boot
    module's ctypes implementation so trace=True can capture NTFF profiles."""
    import types

    if "antenv.axon_hooks" in sys.modules:
        return
    try:
        from trn_agent_boot.trn_boot import _ntff_profile_via_ctypes

        hook = _ntff_profile_via_ctypes("/opt/axon/libaxon_pjrt.so")
        mod = types.ModuleType("antenv.axon_hooks")
        mod.get_axon_ntff_profile_hook = lambda: hook
        mod.set_axon_ntff_profile_hook = lambda h: None
        sys.modules["antenv.axon_hooks"] = mod
        # don't try to copy artifacts to a remote bucket from the sandbox
        import concourse.bass_utils as _bu

        _bu.upload_artifacts = lambda tmpdir: tmpdir
    except Exception as e:  # profiling is best-effort
        print(f"profile shim unavailable: {e}", file=sys.stderr)


def run(q, k, v, mask, trace=False, **kw):
    nc = _get_nc()
    if trace:
        _install_profile_shim()
    in_maps = _make_in_maps(q, k, v, mask)
    res = run_bass_kernel_spmd(nc, in_maps, list(range(NCORES)), trace=trace, **kw)
    return _gather(res.results), res


def kernel(q, k, v, mask):
    out, _ = run(q, k, v, mask)
    return out
